# revision 11
# baseline (speedup 1.0000x reference)
"""Trainium2 Bass kernel for nn_MultiHeadAttention (B=4, S=2048, D=1024, H=16, DH=64).

Sharding: 8 cores = 4 batches x 2 query-halves. Each core computes, for its
(batch b, query half): Q/K/V projections, masked softmax attention over the
full key length, and the output projection, entirely on-device.

Device-side layout is fully transposed (feature-major) so every matmul has its
contraction on the partition dim:
  QT = Wq^T xqT / 8         [HDH, Sq]   (1/8 score scale + bq folded in)
  KT = Wk^T xkT             [HDH, S]    (spilled to DRAM, per-pair reload)
  V  = (Wv^T xvT)^T         [S, HDH]    (computed directly in [s, hdh] layout)
  scoresT_h = KT_h^T QT_h   [S, Sq]     (per head; 2 heads packed in PE rows)
  expT = exp(scoresT + mask_bias[k])    (no row-max: |scores| <= ~4)
  outT_h = V_h^T expT / l   [DH, Sq]    (l = sum_k expT via ones-matmul)
  yT = Wo^T outT + bo'      [D, Sq]
Key-padding mask enters as a per-partition bias (0 / -40) on the Exp
activation. bk is dropped (softmax-invariant); bv,bo fold into bo' = bv@Wo+bo
host-side (exact, since softmax rows sum to 1).
"""

import os
import sys
import numpy as np

DEBUG_DUMP = bool(os.environ.get("KDBG"))

if "/opt/trn_rl_repo" not in sys.path:
    sys.path.insert(0, "/opt/trn_rl_repo")

import concourse.bass as bass
import concourse.mybir as mybir
import concourse.tile as tile
from concourse import bacc
from concourse.bass_utils import run_bass_kernel_spmd

B, S, D = 4, 2048, 1024
H, DH = 16, 64
HDH = H * DH                      # 1024
SQ = S // 2                       # 1024 queries per core
P = 128
DC = D // P                       # 8 contraction chunks
NJ = 8                            # head pairs (2 heads x 64 = 128 rows each)
KC = S // P                       # 16 key chunks
SC = S // P                       # 16 s chunks for V
F32 = mybir.dt.float32
F32R = mybir.dt.float32r
MASK_NEG = -40.0

_CACHE = {}


def build_bass():
    nc = bacc.Bacc("TRN2", target_bir_lowering=False, debug=False)

    xqT = nc.dram_tensor("xqT", [D, SQ], F32R, kind="ExternalInput").ap()
    xkT = nc.dram_tensor("xkT", [D, S], F32R, kind="ExternalInput").ap()
    xvT = nc.dram_tensor("xvT", [D, S], F32R, kind="ExternalInput").ap()
    wq = nc.dram_tensor("wq", [D, HDH], F32R, kind="ExternalInput").ap()
    wk = nc.dram_tensor("wk", [D, HDH], F32R, kind="ExternalInput").ap()
    wv = nc.dram_tensor("wv", [D, HDH], F32R, kind="ExternalInput").ap()
    wo = nc.dram_tensor("wo", [HDH, D], F32R, kind="ExternalInput").ap()
    bq8 = nc.dram_tensor("bq8", [P, NJ], F32, kind="ExternalInput").ap()
    bo2 = nc.dram_tensor("bo2", [P, DC], F32, kind="ExternalInput").ap()
    maskb = nc.dram_tensor("maskb", [P, KC], F32, kind="ExternalInput").ap()
    onesc = nc.dram_tensor("onesc", [P, 1], F32R, kind="ExternalInput").ap()
    yT = nc.dram_tensor("yT", [D, SQ], F32, kind="ExternalOutput").ap()
    dbg = {}
    if DEBUG_DUMP:
        for nm, shp in [("qt0", [P, SQ]), ("v0", [P, HDH]), ("kt0", [P, S]),
                        ("lpA0", [P, SQ]), ("L0", [P, SQ]), ("ot0", [P, SQ]),
                        ("et0", [P, 2 * SQ])]:
            dbg[nm] = nc.dram_tensor("dbg_" + nm, shp, F32, kind="ExternalOutput").ap()

    Exp = mybir.ActivationFunctionType.Exp
    AOp = mybir.AluOpType

    with tile.TileContext(nc) as tc:
        # ---- persistent pools -------------------------------------------
        with (
            tc.tile_pool(name="const", bufs=1) as cpool,
            tc.tile_pool(name="vres", bufs=1) as vpool,
            tc.tile_pool(name="ktdram", bufs=1, space="DRAM") as ktd,
            tc.tile_pool(name="rdram", bufs=2, space="DRAM") as rdp,
        ):
            maskb_sb = cpool.tile([P, KC], F32)
            nc.sync.dma_start(out=maskb_sb, in_=maskb)
            bq8_sb = cpool.tile([P, NJ], F32)
            nc.sync.dma_start(out=bq8_sb, in_=bq8)
            bo2_sb = cpool.tile([P, DC], F32)
            nc.sync.dma_start(out=bo2_sb, in_=bo2)
            ones_sb = cpool.tile([P, 1], F32R)
            nc.sync.dma_start(out=ones_sb, in_=onesc)

            v_sb = vpool.tile([P, SC, HDH], F32R)      # V[s, hdh]; s = sc*128+p
            kt_dram = [ktd.tile([P, S], F32R, tag=f"ktd{j}", name=f"ktd{j}") for j in range(NJ)]

            # ---- phase V: V = (Wv^T xvT)^T, all pairs --------------------
            with (
                tc.tile_pool(name="xv", bufs=1) as xvp,
                tc.tile_pool(name="wvp", bufs=1) as wvp,
                tc.tile_pool(name="pv", bufs=2, space="PSUM") as pvp,
            ):
                xv_t = []
                xv_ch = xvT.rearrange("(c p) s -> c p s", p=P)
                for kc in range(DC):
                    t = xvp.tile([P, S], F32R, tag=f"xv{kc}", name=f"xv{kc}")
                    nc.sync.dma_start(out=t, in_=xv_ch[kc])
                    xv_t.append(t)
                wv_sb = wvp.tile([P, DC, HDH], F32R)
                nc.sync.dma_start(
                    out=wv_sb, in_=wv.rearrange("(c p) n -> p c n", p=P)
                )
                for sc in range(SC):
                    ps = pvp.tile([P, HDH], F32, tag="pv")
                    for kc in range(DC):
                        lhsT = xv_t[kc][:, sc * P:(sc + 1) * P]
                        for nh in range(2):
                            nc.tensor.matmul(
                                ps[:, nh * 512:(nh + 1) * 512],
                                lhsT,
                                wv_sb[:, kc, nh * 512:(nh + 1) * 512],
                                start=(kc == 0),
                                stop=(kc == DC - 1),
                            )
                    nc.vector.tensor_copy(v_sb[:, sc, :], ps)
                    if DEBUG_DUMP and sc == 0:
                        nc.gpsimd.dma_start(out=dbg["v0"], in_=v_sb[:, 0, :])

            # ---- phase K: KT -> DRAM, all pairs --------------------------
            with (
                tc.tile_pool(name="xk", bufs=1) as xkp,
                tc.tile_pool(name="wkp", bufs=1) as wkp,
                tc.tile_pool(name="ktst", bufs=3) as ktstp,
                tc.tile_pool(name="pk", bufs=2, space="PSUM") as pkp,
            ):
                xk_t = []
                xk_ch = xkT.rearrange("(c p) s -> c p s", p=P)
                for kc in range(DC):
                    t = xkp.tile([P, S], F32R, tag=f"xk{kc}", name=f"xk{kc}")
                    nc.sync.dma_start(out=t, in_=xk_ch[kc])
                    xk_t.append(t)
                wk_sb = wkp.tile([P, DC, HDH], F32R)
                nc.sync.dma_start(
                    out=wk_sb, in_=wk.rearrange("(c p) n -> p c n", p=P)
                )
                for j in range(NJ):
                    for half in range(2):
                        ps = pkp.tile([P, SQ], F32, tag="pk")
                        for kc in range(DC):
                            lhsT = wk_sb[:, kc, j * P:(j + 1) * P]
                            for nh in range(2):
                                o = half * SQ + nh * 512
                                nc.tensor.matmul(
                                    ps[:, nh * 512:(nh + 1) * 512],
                                    lhsT,
                                    xk_t[kc][:, o:o + 512],
                                    start=(kc == 0),
                                    stop=(kc == DC - 1),
                                )
                        st = ktstp.tile([P, SQ], F32R, tag="ktst")
                        nc.vector.tensor_copy(st, ps)
                        nc.gpsimd.dma_start(
                            out=kt_dram[j][:, half * SQ:(half + 1) * SQ], in_=st
                        )

            # ---- phase Q: QT resident, all pairs -------------------------
            with tc.tile_pool(name="qtres", bufs=1) as qtpool:
                qt = [qtpool.tile([P, SQ], F32R, tag=f"qt{j}", name=f"qt{j}") for j in range(NJ)]
                with (
                    tc.tile_pool(name="xq", bufs=1) as xqp,
                    tc.tile_pool(name="wqp", bufs=1) as wqp,
                    tc.tile_pool(name="pq", bufs=2, space="PSUM") as pqp,
                ):
                    xq_sb = xqp.tile([P, DC, SQ], F32R)
                    nc.sync.dma_start(
                        out=xq_sb, in_=xqT.rearrange("(c p) s -> p c s", p=P)
                    )
                    wq_sb = wqp.tile([P, DC, HDH], F32R)
                    nc.sync.dma_start(
                        out=wq_sb, in_=wq.rearrange("(c p) n -> p c n", p=P)
                    )
                    for j in range(NJ):
                        ps = pqp.tile([P, SQ], F32, tag="pq")
                        for kc in range(DC):
                            lhsT = wq_sb[:, kc, j * P:(j + 1) * P]
                            for nh in range(2):
                                nc.tensor.matmul(
                                    ps[:, nh * 512:(nh + 1) * 512],
                                    lhsT,
                                    xq_sb[:, kc, nh * 512:(nh + 1) * 512],
                                    start=(kc == 0),
                                    stop=(kc == DC - 1),
                                )
                        nc.vector.tensor_scalar(
                            qt[j], ps, 0.125, bq8_sb[:, j:j + 1],
                            AOp.mult, AOp.add,
                        )
                        if DEBUG_DUMP and j == 0:
                            nc.gpsimd.dma_start(out=dbg["qt0"], in_=qt[0])

                # ---- attention + output accumulation ---------------------
                with tc.tile_pool(name="otres", bufs=1) as otpool:
                    ot = [
                        otpool.tile([P, SQ], F32R, tag=f"ot{j}", name=f"ot{j}")
                        for j in range(NJ)
                    ]
                    with (
                        tc.tile_pool(name="ktsb", bufs=2) as ktp,
                        tc.tile_pool(name="expp", bufs=2) as expp,
                        tc.tile_pool(name="lp", bufs=1) as lpp,
                        tc.tile_pool(name="lbp", bufs=2) as lbp,
                        tc.tile_pool(name="rp", bufs=2) as rpp,
                        tc.tile_pool(name="ps_s", bufs=1, space="PSUM") as pss,
                        tc.tile_pool(name="ps_o", bufs=1, space="PSUM") as pso,
                    ):
                        for j in range(NJ):
                            kt_sb = ktp.tile([P, S], F32R, tag="kt")
                            nc.sync.dma_start(out=kt_sb, in_=kt_dram[j][:])
                            if DEBUG_DUMP and j == 0:
                                nc.gpsimd.dma_start(out=dbg["kt0"], in_=kt_sb)
                            lpA = lpp.tile([P, SQ], F32R, tag="lpA")
                            lpB = lpp.tile([P, SQ], F32R, tag="lpB")
                            ps_oa = pso.tile([64, SQ], F32, tag="oa")
                            ps_ob = pso.tile([64, SQ], F32, tag="ob")
                            for kc in range(KC):
                                ps_s = pss.tile([P, 2 * SQ], F32, tag="s")
                                for hh in range(2):
                                    lhsT = kt_sb[hh * 64:(hh + 1) * 64,
                                                 kc * P:(kc + 1) * P]
                                    rhs_t = qt[j]
                                    for nh in range(2):
                                        nc.tensor.matmul(
                                            ps_s[:, hh * SQ + nh * 512:
                                                 hh * SQ + (nh + 1) * 512],
                                            lhsT,
                                            rhs_t[hh * 64:(hh + 1) * 64,
                                                  nh * 512:(nh + 1) * 512],
                                            tile_position=(hh * 64, 0),
                                        )
                                et = expp.tile([P, 2 * SQ], F32R, tag="e")
                                nc.scalar.activation(
                                    et, ps_s, Exp,
                                    bias=maskb_sb[:, kc:kc + 1], scale=1.0,
                                )
                                if DEBUG_DUMP and j == 0 and kc == 0:
                                    nc.gpsimd.dma_start(out=dbg["et0"], in_=et)
                                if kc == 0:
                                    nc.vector.tensor_copy(lpA, et[:, 0:SQ])
                                    nc.vector.tensor_copy(lpB, et[:, SQ:2 * SQ])
                                else:
                                    nc.vector.tensor_add(lpA, lpA, et[:, 0:SQ])
                                    nc.vector.tensor_add(
                                        lpB, lpB, et[:, SQ:2 * SQ]
                                    )
                                for hh, ps_o in ((0, ps_oa), (1, ps_ob)):
                                    lhsT = v_sb[:, kc,
                                             j * P + hh * 64:
                                             j * P + (hh + 1) * 64]
                                    for nh in range(2):
                                        nc.tensor.matmul(
                                            ps_o[:, nh * 512:(nh + 1) * 512],
                                            lhsT,
                                            et[:, hh * SQ + nh * 512:
                                                  hh * SQ + (nh + 1) * 512],
                                            start=(kc == 0),
                                            stop=(kc == KC - 1),
                                        )
                            # normalization: l = ones^T lp; ot = ps_o * (1/l)
                            L_sb = lbp.tile([P, SQ], F32, tag="L")
                            for hh, lp_t in ((0, lpA), (1, lpB)):
                                ps_l = pss.tile([1, SQ], F32, tag="s", name="ps_l")
                                for nh in range(2):
                                    nc.tensor.matmul(
                                        ps_l[:, nh * 512:(nh + 1) * 512],
                                        ones_sb,
                                        lp_t[:, nh * 512:(nh + 1) * 512],
                                    )
                                rr = rpp.tile([1, SQ], F32, tag=f"r{hh}", name=f"rr{hh}")
                                nc.vector.reciprocal(rr, ps_l)
                                rd = rdp.tile([1, SQ], F32, tag="rd", name="rd")
                                nc.gpsimd.dma_start(out=rd, in_=rr)
                                rd_b = bass.AP(
                                    tensor=rd.tensor, offset=rd.offset,
                                    ap=[[0, 64], rd.ap[-1]],
                                )
                                nc.sync.dma_start(
                                    out=L_sb[hh * 64:(hh + 1) * 64, :], in_=rd_b
                                )
                            if DEBUG_DUMP and j == 0:
                                nc.gpsimd.dma_start(out=dbg["lpA0"], in_=lpA)
                                nc.gpsimd.dma_start(out=dbg["L0"], in_=L_sb)
                            nc.vector.tensor_mul(
                                ot[j][0:64, :], ps_oa, L_sb[0:64, :]
                            )
                            tmpb = lbp.tile([64, SQ], F32R, tag="tmpb")
                            nc.vector.tensor_copy(tmpb, ps_ob)
                            nc.gpsimd.dma_start(
                                out=ot[j][64:128, :], in_=tmpb
                            )
                            nc.vector.tensor_mul(
                                ot[j][64:128, :], ot[j][64:128, :],
                                L_sb[64:128, :],
                            )
                            if DEBUG_DUMP and j == 0:
                                nc.gpsimd.dma_start(out=dbg["ot0"], in_=ot[0])

                    # ---- output projection ---------------------------
                    with (
                        tc.tile_pool(name="wop", bufs=4) as wop,
                        tc.tile_pool(name="ytp", bufs=3) as ytp,
                        tc.tile_pool(name="py", bufs=2, space="PSUM") as pyp,
                    ):
                        yt_ch = yT.rearrange("(c p) s -> c p s", p=P)
                        for dc in range(DC):
                            ps = pyp.tile([P, SQ], F32, tag="py")
                            for j in range(NJ):
                                wo_t = wop.tile([P, P], F32R, tag="wo")
                                nc.sync.dma_start(
                                    out=wo_t,
                                    in_=wo[j * P:(j + 1) * P,
                                           dc * P:(dc + 1) * P],
                                )
                                for nh in range(2):
                                    nc.tensor.matmul(
                                        ps[:, nh * 512:(nh + 1) * 512],
                                        wo_t,
                                        ot[j][:, nh * 512:(nh + 1) * 512],
                                        start=(j == 0),
                                        stop=(j == NJ - 1),
                                    )
                            yt_sb = ytp.tile([P, SQ], F32, tag="yt")
                            nc.vector.tensor_scalar(
                                yt_sb, ps, bo2_sb[:, dc:dc + 1], None, AOp.add
                            )
                            nc.gpsimd.dma_start(out=yt_ch[dc], in_=yt_sb)

    nc.compile()
    return nc


def kernel(x_Q, x_K, x_V, src_batch_lens, Wq, bq, Wk, bk, Wv, bv, Wo, bo):
    x_Q = np.asarray(x_Q, dtype=np.float32)
    x_K = np.asarray(x_K, dtype=np.float32)
    x_V = np.asarray(x_V, dtype=np.float32)
    lens = np.asarray(src_batch_lens)
    Wq = np.ascontiguousarray(np.asarray(Wq, dtype=np.float32))
    Wk = np.ascontiguousarray(np.asarray(Wk, dtype=np.float32))
    Wv = np.ascontiguousarray(np.asarray(Wv, dtype=np.float32))
    Wo = np.ascontiguousarray(np.asarray(Wo, dtype=np.float32))
    bq = np.asarray(bq, dtype=np.float32)
    bv = np.asarray(bv, dtype=np.float32)
    bo = np.asarray(bo, dtype=np.float32)

    if "nc" not in _CACHE:
        _CACHE["nc"] = build_bass()
    nc = _CACHE["nc"]

    bo2_full = (bv @ Wo + bo).astype(np.float32)
    bo2 = np.ascontiguousarray(bo2_full.reshape(DC, P).T)
    bq8 = np.ascontiguousarray((bq / 8.0).reshape(NJ, P).T)

    in_maps = []
    for c in range(8):
        b, hh = c // 2, c % 2
        q0 = hh * SQ
        k_idx = np.arange(S)
        mvec = np.where(k_idx < int(lens[b]), 0.0, MASK_NEG).astype(np.float32)
        in_maps.append({
            "xqT": np.ascontiguousarray(x_Q[b, q0:q0 + SQ, :].T),
            "xkT": np.ascontiguousarray(x_K[b].T),
            "xvT": np.ascontiguousarray(x_V[b].T),
            "wq": Wq, "wk": Wk, "wv": Wv, "wo": Wo,
            "bq8": bq8, "bo2": bo2,
            "onesc": np.ones((P, 1), np.float32),
            "maskb": np.ascontiguousarray(mvec.reshape(KC, P).T),
        })

    res = run_bass_kernel_spmd(nc, in_maps, core_ids=list(range(8)))

    out = np.empty((B, S, D), dtype=np.float32)
    for c in range(8):
        b, hh = c // 2, c % 2
        q0 = hh * SQ
        out[b, q0:q0 + SQ, :] = res.results[c]["yT"].T
    return out


# revision 13
# speedup vs baseline: 1.2146x; 1.2146x over previous
"""Trainium2 Bass kernel for nn_MultiHeadAttention (B=4, S=2048, D=1024, H=16, DH=64).

Sharding: 8 cores = 4 batches x 2 query-halves. Each core computes, for its
(batch b, query half): Q/K/V projections, masked softmax attention over the
full key length, and the output projection, entirely on-device.

Device-side layout is fully transposed (feature-major) so every matmul has its
contraction on the partition dim:
  QT = Wq^T xqT / 8         [HDH, Sq]   (1/8 score scale + bq folded in)
  KT = Wk^T xkT             [HDH, S]    (spilled to DRAM, per-pair reload)
  V  = (Wv^T xvT)^T         [S, HDH]    stored interleaved per head as
                                        [s, h, 65] with a ones column, so the
                                        PV matmul's PSUM row 64 accumulates
                                        l = sum_k exp(scoresT) for free
  scoresT_h = KT_h^T QT_h   [S, Sq]     (per head; 2 heads packed in PE rows)
  expT = exp(scoresT + mask_bias[k])    (no row-max: |scores| <= ~4)
  outT_h = V_h^T expT / l   [DH, Sq]
  yT = Wo^T outT + bo'      [D, Sq]
Key-padding mask enters as a per-partition bias (0 / -40) on the Exp
activation. bk is dropped (softmax-invariant); bv,bo fold into bo' = bv@Wo+bo
host-side (exact, since softmax rows sum to 1).
"""

import os
import sys
import numpy as np

if "/opt/trn_rl_repo" not in sys.path:
    sys.path.insert(0, "/opt/trn_rl_repo")

import concourse.bass as bass
import concourse.mybir as mybir
import concourse.tile as tile
from concourse import bacc
from concourse.bass_utils import run_bass_kernel_spmd

B, S, D = 4, 2048, 1024
H, DH = 16, 64
HDH = H * DH                      # 1024
SQ = S // 2                       # 1024 queries per core
P = 128
DC = D // P                       # 8 contraction chunks
NJ = 8                            # head pairs (2 heads x 64 rows = 128)
KC = S // P                       # 16 key chunks
SC = S // P                       # 16 s chunks for V
VW = DH + 1                       # 65: V columns per head + ones column
F32 = mybir.dt.float32
F32R = mybir.dt.float32r
MASK_NEG = -40.0

_CACHE = {}


def build_bass():
    nc = bacc.Bacc("TRN2", target_bir_lowering=False, debug=False)

    xqT = nc.dram_tensor("xqT", [D, SQ], F32R, kind="ExternalInput").ap()
    xkT = nc.dram_tensor("xkT", [D, S], F32R, kind="ExternalInput").ap()
    xvT = nc.dram_tensor("xvT", [D, S], F32R, kind="ExternalInput").ap()
    wq = nc.dram_tensor("wq", [D, HDH], F32R, kind="ExternalInput").ap()
    wk = nc.dram_tensor("wk", [D, HDH], F32R, kind="ExternalInput").ap()
    wv = nc.dram_tensor("wv", [D, HDH], F32R, kind="ExternalInput").ap()
    wo = nc.dram_tensor("wo", [HDH, D], F32R, kind="ExternalInput").ap()
    bq8 = nc.dram_tensor("bq8", [P, NJ], F32, kind="ExternalInput").ap()
    bo2 = nc.dram_tensor("bo2", [P, DC], F32, kind="ExternalInput").ap()
    maskb = nc.dram_tensor("maskb", [P, KC], F32, kind="ExternalInput").ap()
    vones = nc.dram_tensor("vones", [P, SC * H], F32R, kind="ExternalInput").ap()
    yT = nc.dram_tensor("yT", [D, SQ], F32, kind="ExternalOutput").ap()

    Exp = mybir.ActivationFunctionType.Exp
    AOp = mybir.AluOpType

    with tile.TileContext(nc) as tc:
        with (
            tc.tile_pool(name="const", bufs=1) as cpool,
            tc.tile_pool(name="vres", bufs=1) as vpool,
            tc.tile_pool(name="ktdram", bufs=1, space="DRAM") as ktd,
            tc.tile_pool(name="rdram", bufs=2, space="DRAM") as rdp,
        ):
            maskb_sb = cpool.tile([P, KC], F32)
            nc.sync.dma_start(out=maskb_sb, in_=maskb)
            bq8_sb = cpool.tile([P, NJ], F32)
            nc.sync.dma_start(out=bq8_sb, in_=bq8)
            bo2_sb = cpool.tile([P, DC], F32)
            nc.sync.dma_start(out=bo2_sb, in_=bo2)

            # V interleaved per head: [p, sc, h, 65]; col 64 of each head = 1.0
            v_sb = vpool.tile([P, SC, H, VW], F32R)
            nc.sync.dma_start(
                out=v_sb[:, :, :, DH:DH + 1],
                in_=vones.rearrange("p (sc h one) -> p sc h one", h=H, one=1),
            )
            kt_dram = [
                ktd.tile([P, S], F32R, tag=f"ktd{j}", name=f"ktd{j}")
                for j in range(NJ)
            ]

            # ---- phase V: V = (Wv^T xvT)^T, all pairs --------------------
            with (
                tc.tile_pool(name="xv", bufs=1) as xvp,
                tc.tile_pool(name="wvp", bufs=1) as wvp,
                tc.tile_pool(name="pv", bufs=2, space="PSUM") as pvp,
            ):
                xv_t = []
                xv_ch = xvT.rearrange("(c p) s -> c p s", p=P)
                for kc in range(DC):
                    t = xvp.tile([P, S], F32R, tag=f"xv{kc}", name=f"xv{kc}")
                    nc.sync.dma_start(out=t, in_=xv_ch[kc])
                    xv_t.append(t)
                wv_sb = wvp.tile([P, DC, HDH], F32R)
                nc.sync.dma_start(
                    out=wv_sb, in_=wv.rearrange("(c p) n -> p c n", p=P)
                )
                for sc in range(SC):
                    ps = pvp.tile([P, HDH], F32, tag="pv")
                    for kc in range(DC):
                        lhsT = xv_t[kc][:, sc * P:(sc + 1) * P]
                        for nh in range(2):
                            nc.tensor.matmul(
                                ps[:, nh * 512:(nh + 1) * 512],
                                lhsT,
                                wv_sb[:, kc, nh * 512:(nh + 1) * 512],
                                start=(kc == 0),
                                stop=(kc == DC - 1),
                            )
                    # strided copy into the interleaved layout
                    nc.vector.tensor_copy(
                        v_sb[:, sc, :, 0:DH],
                        ps.rearrange("p (h d) -> p h d", d=DH),
                    )

            # ---- phase K: KT -> DRAM, all pairs --------------------------
            with (
                tc.tile_pool(name="xk", bufs=1) as xkp,
                tc.tile_pool(name="wkp", bufs=1) as wkp,
                tc.tile_pool(name="ktst", bufs=3) as ktstp,
                tc.tile_pool(name="pk", bufs=2, space="PSUM") as pkp,
            ):
                xk_t = []
                xk_ch = xkT.rearrange("(c p) s -> c p s", p=P)
                for kc in range(DC):
                    t = xkp.tile([P, S], F32R, tag=f"xk{kc}", name=f"xk{kc}")
                    nc.sync.dma_start(out=t, in_=xk_ch[kc])
                    xk_t.append(t)
                wk_sb = wkp.tile([P, DC, HDH], F32R)
                nc.sync.dma_start(
                    out=wk_sb, in_=wk.rearrange("(c p) n -> p c n", p=P)
                )
                for j in range(NJ):
                    for half in range(2):
                        ps = pkp.tile([P, SQ], F32, tag="pk")
                        for kc in range(DC):
                            lhsT = wk_sb[:, kc, j * P:(j + 1) * P]
                            for nh in range(2):
                                o = half * SQ + nh * 512
                                nc.tensor.matmul(
                                    ps[:, nh * 512:(nh + 1) * 512],
                                    lhsT,
                                    xk_t[kc][:, o:o + 512],
                                    start=(kc == 0),
                                    stop=(kc == DC - 1),
                                )
                        st = ktstp.tile([P, SQ], F32R, tag="ktst")
                        nc.vector.tensor_copy(st, ps)
                        nc.gpsimd.dma_start(
                            out=kt_dram[j][:, half * SQ:(half + 1) * SQ],
                            in_=st,
                        )

            # ---- phase Q: QT resident, all pairs -------------------------
            with tc.tile_pool(name="qtres", bufs=1) as qtpool:
                qt = [
                    qtpool.tile([P, SQ], F32R, tag=f"qt{j}", name=f"qt{j}")
                    for j in range(NJ)
                ]
                with (
                    tc.tile_pool(name="xq", bufs=1) as xqp,
                    tc.tile_pool(name="wqp", bufs=1) as wqp,
                    tc.tile_pool(name="pq", bufs=2, space="PSUM") as pqp,
                ):
                    xq_sb = xqp.tile([P, DC, SQ], F32R)
                    nc.sync.dma_start(
                        out=xq_sb, in_=xqT.rearrange("(c p) s -> p c s", p=P)
                    )
                    wq_sb = wqp.tile([P, DC, HDH], F32R)
                    nc.sync.dma_start(
                        out=wq_sb, in_=wq.rearrange("(c p) n -> p c n", p=P)
                    )
                    for j in range(NJ):
                        ps = pqp.tile([P, SQ], F32, tag="pq")
                        for kc in range(DC):
                            lhsT = wq_sb[:, kc, j * P:(j + 1) * P]
                            for nh in range(2):
                                nc.tensor.matmul(
                                    ps[:, nh * 512:(nh + 1) * 512],
                                    lhsT,
                                    xq_sb[:, kc, nh * 512:(nh + 1) * 512],
                                    start=(kc == 0),
                                    stop=(kc == DC - 1),
                                )
                        nc.vector.tensor_scalar(
                            qt[j], ps, 0.125, bq8_sb[:, j:j + 1],
                            AOp.mult, AOp.add,
                        )

                # ---- attention -------------------------------------------
                with tc.tile_pool(name="otres", bufs=1) as otpool:
                    ot = [
                        otpool.tile([P, SQ], F32R, tag=f"ot{j}", name=f"ot{j}")
                        for j in range(NJ)
                    ]
                    with (
                        tc.tile_pool(name="ktsb", bufs=2) as ktp,
                        tc.tile_pool(name="expp", bufs=3) as expp,
                        tc.tile_pool(name="lbp", bufs=2) as lbp,
                        tc.tile_pool(name="rp", bufs=1) as rpp,
                        tc.tile_pool(name="ps_s", bufs=2, space="PSUM") as pss,
                        tc.tile_pool(name="ps_o", bufs=1, space="PSUM") as pso,
                    ):
                        for j in range(NJ):
                            kt_sb = ktp.tile([P, S], F32R, tag="kt")
                            nc.sync.dma_start(out=kt_sb, in_=kt_dram[j][:])
                            ps_oa = pso.tile([VW, SQ], F32, tag="oa")
                            ps_ob = pso.tile([VW, SQ], F32, tag="ob")
                            for kc in range(KC):
                                for hh, ps_o in ((0, ps_oa), (1, ps_ob)):
                                    ps_s = pss.tile(
                                        [P, SQ], F32, tag="s", name="ps_s"
                                    )
                                    lhsT = kt_sb[hh * 64:(hh + 1) * 64,
                                                 kc * P:(kc + 1) * P]
                                    for nh in range(2):
                                        nc.tensor.matmul(
                                            ps_s[:, nh * 512:(nh + 1) * 512],
                                            lhsT,
                                            qt[j][hh * 64:(hh + 1) * 64,
                                                  nh * 512:(nh + 1) * 512],
                                            tile_position=(hh * 64, 0),
                                        )
                                    et = expp.tile(
                                        [P, SQ], F32R, tag="e", name="et"
                                    )
                                    nc.scalar.activation(
                                        et, ps_s, Exp,
                                        bias=maskb_sb[:, kc:kc + 1], scale=1.0,
                                    )
                                    vh = v_sb[:, kc, 2 * j + hh, :]  # [128,65]
                                    for nh in range(2):
                                        nc.tensor.matmul(
                                            ps_o[:, nh * 512:(nh + 1) * 512],
                                            vh,
                                            et[:, nh * 512:(nh + 1) * 512],
                                            start=(kc == 0),
                                            stop=(kc == KC - 1),
                                        )
                            # normalize: row 64 of ps_o* is l
                            rrow = rpp.tile([VW, 2, SQ], F32, tag="rr")
                            nc.vector.reciprocal(
                                rrow[DH:VW, 0, :], ps_oa[DH:VW, :]
                            )
                            nc.vector.reciprocal(
                                rrow[DH:VW, 1, :], ps_ob[DH:VW, :]
                            )
                            L_sb = lbp.tile([P, SQ], F32, tag="L")
                            for hh in range(2):
                                rd = rdp.tile([1, SQ], F32, tag="rd", name="rd")
                                nc.gpsimd.dma_start(
                                    out=rd, in_=rrow[DH:VW, hh, :]
                                )
                                rd_b = bass.AP(
                                    tensor=rd.tensor, offset=rd.offset,
                                    ap=[[0, 64], rd.ap[-1]],
                                )
                                nc.sync.dma_start(
                                    out=L_sb[hh * 64:(hh + 1) * 64, :],
                                    in_=rd_b,
                                )
                            nc.vector.tensor_mul(
                                ot[j][0:64, :], ps_oa[0:DH, :], L_sb[0:64, :]
                            )
                            tmpb = lbp.tile([64, SQ], F32R, tag="tmpb")
                            nc.vector.tensor_copy(tmpb, ps_ob[0:DH, :])
                            nc.gpsimd.dma_start(
                                out=ot[j][64:128, :], in_=tmpb
                            )
                            nc.vector.tensor_mul(
                                ot[j][64:128, :], ot[j][64:128, :],
                                L_sb[64:128, :],
                            )

                    # ---- output projection -------------------------------
                    with (
                        tc.tile_pool(name="wop", bufs=4) as wop,
                        tc.tile_pool(name="ytp", bufs=3) as ytp,
                        tc.tile_pool(name="py", bufs=2, space="PSUM") as pyp,
                    ):
                        yt_ch = yT.rearrange("(c p) s -> c p s", p=P)
                        for dc in range(DC):
                            ps = pyp.tile([P, SQ], F32, tag="py")
                            for j in range(NJ):
                                wo_t = wop.tile([P, P], F32R, tag="wo")
                                nc.sync.dma_start(
                                    out=wo_t,
                                    in_=wo[j * P:(j + 1) * P,
                                           dc * P:(dc + 1) * P],
                                )
                                for nh in range(2):
                                    nc.tensor.matmul(
                                        ps[:, nh * 512:(nh + 1) * 512],
                                        wo_t,
                                        ot[j][:, nh * 512:(nh + 1) * 512],
                                        start=(j == 0),
                                        stop=(j == NJ - 1),
                                    )
                            yt_sb = ytp.tile([P, SQ], F32, tag="yt")
                            nc.vector.tensor_scalar(
                                yt_sb, ps, bo2_sb[:, dc:dc + 1], None, AOp.add
                            )
                            nc.gpsimd.dma_start(out=yt_ch[dc], in_=yt_sb)

    nc.compile()
    return nc


def kernel(x_Q, x_K, x_V, src_batch_lens, Wq, bq, Wk, bk, Wv, bv, Wo, bo):
    x_Q = np.asarray(x_Q, dtype=np.float32)
    x_K = np.asarray(x_K, dtype=np.float32)
    x_V = np.asarray(x_V, dtype=np.float32)
    lens = np.asarray(src_batch_lens)
    Wq = np.ascontiguousarray(np.asarray(Wq, dtype=np.float32))
    Wk = np.ascontiguousarray(np.asarray(Wk, dtype=np.float32))
    Wv = np.ascontiguousarray(np.asarray(Wv, dtype=np.float32))
    Wo = np.ascontiguousarray(np.asarray(Wo, dtype=np.float32))
    bq = np.asarray(bq, dtype=np.float32)
    bv = np.asarray(bv, dtype=np.float32)
    bo = np.asarray(bo, dtype=np.float32)

    if "nc" not in _CACHE:
        _CACHE["nc"] = build_bass()
    nc = _CACHE["nc"]

    bo2_full = (bv @ Wo + bo).astype(np.float32)
    bo2 = np.ascontiguousarray(bo2_full.reshape(DC, P).T)
    bq8 = np.ascontiguousarray((bq / 8.0).reshape(NJ, P).T)

    in_maps = []
    for c in range(8):
        b, hh = c // 2, c % 2
        q0 = hh * SQ
        k_idx = np.arange(S)
        mvec = np.where(k_idx < int(lens[b]), 0.0, MASK_NEG).astype(np.float32)
        in_maps.append({
            "xqT": np.ascontiguousarray(x_Q[b, q0:q0 + SQ, :].T),
            "xkT": np.ascontiguousarray(x_K[b].T),
            "xvT": np.ascontiguousarray(x_V[b].T),
            "wq": Wq, "wk": Wk, "wv": Wv, "wo": Wo,
            "bq8": bq8, "bo2": bo2,
            "vones": np.ones((P, SC * H), np.float32),
            "maskb": np.ascontiguousarray(mvec.reshape(KC, P).T),
        })

    res = run_bass_kernel_spmd(nc, in_maps, core_ids=list(range(8)))

    out = np.empty((B, S, D), dtype=np.float32)
    for c in range(8):
        b, hh = c // 2, c % 2
        q0 = hh * SQ
        out[b, q0:q0 + SQ, :] = res.results[c]["yT"].T
    return out


# revision 14
# speedup vs baseline: 1.4006x; 1.1532x over previous
"""Trainium2 Bass kernel for nn_MultiHeadAttention (B=4, S=2048, D=1024, H=16, DH=64).

Sharding: 8 cores = 4 batches x 2 query-halves. Each core computes, for its
(batch b, query half): Q/K/V projections, masked softmax attention over the
full key length, and the output projection, entirely on-device.

Device-side layout is fully transposed (feature-major) so every matmul has its
contraction on the partition dim:
  QT = Wq^T xqT / 8         [HDH, Sq]   (1/8 score scale + bq folded in)
  KT = Wk^T xkT             [HDH, S]    (spilled to DRAM, per-pair reload)
  V  = (Wv^T xvT)^T         [S, HDH]    stored interleaved per head as
                                        [s, h, 65] with a ones column, so the
                                        PV matmul's PSUM row 64 accumulates
                                        l = sum_k exp(scoresT) for free
  scoresT_h = KT_h^T QT_h   [S, Sq]     (per head; 2 heads packed in PE rows)
  expT = exp(scoresT + mask_bias[k])    (no row-max: |scores| <= ~4)
  outT_h = V_h^T expT / l   [DH, Sq]
  yT = Wo^T outT + bo'      [D, Sq]
Key-padding mask enters as a per-partition bias (0 / -40) on the Exp
activation. bk is dropped (softmax-invariant); bv,bo fold into bo' = bv@Wo+bo
host-side (exact, since softmax rows sum to 1).
"""

import os
import sys
import numpy as np

if "/opt/trn_rl_repo" not in sys.path:
    sys.path.insert(0, "/opt/trn_rl_repo")

import concourse.bass as bass
import concourse.mybir as mybir
import concourse.tile as tile
from concourse import bacc
from concourse.bass_utils import run_bass_kernel_spmd

B, S, D = 4, 2048, 1024
H, DH = 16, 64
HDH = H * DH                      # 1024
SQ = S // 2                       # 1024 queries per core
P = 128
DC = D // P                       # 8 contraction chunks
NJ = 8                            # head pairs (2 heads x 64 rows = 128)
KC = S // P                       # 16 key chunks
SC = S // P                       # 16 s chunks for V
VW = DH + 1                       # 65: V columns per head + ones column
F32 = mybir.dt.float32
F32R = mybir.dt.float32r
MASK_NEG = -40.0

_CACHE = {}


def build_bass():
    nc = bacc.Bacc("TRN2", target_bir_lowering=False, debug=False)

    xqT = nc.dram_tensor("xqT", [D, SQ], F32R, kind="ExternalInput").ap()
    xkT = nc.dram_tensor("xkT", [D, S], F32R, kind="ExternalInput").ap()
    xvT = nc.dram_tensor("xvT", [D, S], F32R, kind="ExternalInput").ap()
    wq = nc.dram_tensor("wq", [D, HDH], F32R, kind="ExternalInput").ap()
    wk = nc.dram_tensor("wk", [D, HDH], F32R, kind="ExternalInput").ap()
    wv = nc.dram_tensor("wv", [D, HDH], F32R, kind="ExternalInput").ap()
    wo = nc.dram_tensor("wo", [HDH, D], F32R, kind="ExternalInput").ap()
    bq8 = nc.dram_tensor("bq8", [P, NJ], F32, kind="ExternalInput").ap()
    bo2 = nc.dram_tensor("bo2", [P, DC], F32, kind="ExternalInput").ap()
    maskb = nc.dram_tensor("maskb", [P, KC], F32, kind="ExternalInput").ap()
    vones = nc.dram_tensor("vones", [P, SC * H], F32R, kind="ExternalInput").ap()
    yT = nc.dram_tensor("yT", [D, SQ], F32, kind="ExternalOutput").ap()

    Exp = mybir.ActivationFunctionType.Exp
    AOp = mybir.AluOpType

    with tile.TileContext(nc) as tc:
        with (
            tc.tile_pool(name="const", bufs=1) as cpool,
            tc.tile_pool(name="vres", bufs=1) as vpool,
            tc.tile_pool(name="ktdram", bufs=1, space="DRAM") as ktd,
            tc.tile_pool(name="rdram", bufs=2, space="DRAM") as rdp,
        ):
            maskb_sb = cpool.tile([P, KC], F32)
            nc.sync.dma_start(out=maskb_sb, in_=maskb)
            bq8_sb = cpool.tile([P, NJ], F32)
            nc.sync.dma_start(out=bq8_sb, in_=bq8)
            bo2_sb = cpool.tile([P, DC], F32)
            nc.sync.dma_start(out=bo2_sb, in_=bo2)

            # V interleaved per head: [p, sc, h, 65]; col 64 of each head = 1.0
            v_sb = vpool.tile([P, SC, H, VW], F32R)
            nc.sync.dma_start(
                out=v_sb[:, :, :, DH:DH + 1],
                in_=vones.rearrange("p (sc h one) -> p sc h one", h=H, one=1),
            )
            kt_dram = [
                ktd.tile([P, S], F32R, tag=f"ktd{j}", name=f"ktd{j}")
                for j in range(NJ)
            ]

            # ---- phase V: V = (Wv^T xvT)^T, all pairs --------------------
            with (
                tc.tile_pool(name="xv", bufs=1) as xvp,
                tc.tile_pool(name="wvp", bufs=1) as wvp,
                tc.tile_pool(name="pv", bufs=2, space="PSUM") as pvp,
            ):
                xv_t = []
                xv_ch = xvT.rearrange("(c p) s -> c p s", p=P)
                for kc in range(DC):
                    t = xvp.tile([P, S], F32R, tag=f"xv{kc}", name=f"xv{kc}")
                    eng = nc.sync if kc % 2 == 0 else nc.scalar
                    eng.dma_start(out=t, in_=xv_ch[kc])
                    xv_t.append(t)
                wv_sb = wvp.tile([P, DC, HDH], F32R)
                wv_ch = wv.rearrange("(c p) n -> p c n", p=P)
                for kc in range(DC):
                    eng = nc.scalar if kc % 2 == 0 else nc.sync
                    eng.dma_start(out=wv_sb[:, kc, :], in_=wv_ch[:, kc, :])
                for sc in range(SC):
                    ps = pvp.tile([P, HDH], F32, tag="pv")
                    for kc in range(DC):
                        lhsT = xv_t[kc][:, sc * P:(sc + 1) * P]
                        for nh in range(2):
                            nc.tensor.matmul(
                                ps[:, nh * 512:(nh + 1) * 512],
                                lhsT,
                                wv_sb[:, kc, nh * 512:(nh + 1) * 512],
                                start=(kc == 0),
                                stop=(kc == DC - 1),
                            )
                    # strided copy into the interleaved layout
                    nc.vector.tensor_copy(
                        v_sb[:, sc, :, 0:DH],
                        ps.rearrange("p (h d) -> p h d", d=DH),
                    )

            # ---- phase K: KT -> DRAM, all pairs --------------------------
            with (
                tc.tile_pool(name="xk", bufs=1) as xkp,
                tc.tile_pool(name="wkp", bufs=1) as wkp,
                tc.tile_pool(name="ktst", bufs=3) as ktstp,
                tc.tile_pool(name="pk", bufs=2, space="PSUM") as pkp,
            ):
                xk_t = []
                xk_ch = xkT.rearrange("(c p) s -> c p s", p=P)
                for kc in range(DC):
                    t = xkp.tile([P, S], F32R, tag=f"xk{kc}", name=f"xk{kc}")
                    eng = nc.sync if kc % 2 == 0 else nc.scalar
                    eng.dma_start(out=t, in_=xk_ch[kc])
                    xk_t.append(t)
                wk_sb = wkp.tile([P, DC, HDH], F32R)
                wk_ch = wk.rearrange("(c p) n -> p c n", p=P)
                for kc in range(DC):
                    eng = nc.scalar if kc % 2 == 0 else nc.sync
                    eng.dma_start(out=wk_sb[:, kc, :], in_=wk_ch[:, kc, :])
                for j in range(NJ):
                    for half in range(2):
                        ps = pkp.tile([P, SQ], F32, tag="pk")
                        for kc in range(DC):
                            lhsT = wk_sb[:, kc, j * P:(j + 1) * P]
                            for nh in range(2):
                                o = half * SQ + nh * 512
                                nc.tensor.matmul(
                                    ps[:, nh * 512:(nh + 1) * 512],
                                    lhsT,
                                    xk_t[kc][:, o:o + 512],
                                    start=(kc == 0),
                                    stop=(kc == DC - 1),
                                )
                        st = ktstp.tile([P, SQ], F32R, tag="ktst")
                        nc.vector.tensor_copy(st, ps)
                        nc.gpsimd.dma_start(
                            out=kt_dram[j][:, half * SQ:(half + 1) * SQ],
                            in_=st,
                        )

            # ---- phase Q: QT resident, all pairs -------------------------
            with tc.tile_pool(name="qtres", bufs=1) as qtpool:
                qt = [
                    qtpool.tile([P, SQ], F32R, tag=f"qt{j}", name=f"qt{j}")
                    for j in range(NJ)
                ]
                with (
                    tc.tile_pool(name="xq", bufs=1) as xqp,
                    tc.tile_pool(name="wqp", bufs=1) as wqp,
                    tc.tile_pool(name="pq", bufs=2, space="PSUM") as pqp,
                ):
                    xq_sb = xqp.tile([P, DC, SQ], F32R)
                    xq_ch = xqT.rearrange("(c p) s -> p c s", p=P)
                    wq_sb = wqp.tile([P, DC, HDH], F32R)
                    wq_ch = wq.rearrange("(c p) n -> p c n", p=P)
                    for kc in range(DC):
                        eng = nc.sync if kc % 2 == 0 else nc.scalar
                        eng.dma_start(out=xq_sb[:, kc, :], in_=xq_ch[:, kc, :])
                        eng2 = nc.scalar if kc % 2 == 0 else nc.sync
                        eng2.dma_start(out=wq_sb[:, kc, :], in_=wq_ch[:, kc, :])
                    for j in range(NJ):
                        ps = pqp.tile([P, SQ], F32, tag="pq")
                        for kc in range(DC):
                            lhsT = wq_sb[:, kc, j * P:(j + 1) * P]
                            for nh in range(2):
                                nc.tensor.matmul(
                                    ps[:, nh * 512:(nh + 1) * 512],
                                    lhsT,
                                    xq_sb[:, kc, nh * 512:(nh + 1) * 512],
                                    start=(kc == 0),
                                    stop=(kc == DC - 1),
                                )
                        nc.vector.tensor_scalar(
                            qt[j], ps, 0.125, bq8_sb[:, j:j + 1],
                            AOp.mult, AOp.add,
                        )

                # ---- attention -------------------------------------------
                with tc.tile_pool(name="otres", bufs=1) as otpool:
                    ot = [
                        otpool.tile([P, SQ], F32R, tag=f"ot{j}", name=f"ot{j}")
                        for j in range(NJ)
                    ]
                    with (
                        tc.tile_pool(name="ktsb", bufs=2) as ktp,
                        tc.tile_pool(name="expp", bufs=6) as expp,
                        tc.tile_pool(name="lbp", bufs=2) as lbp,
                        tc.tile_pool(name="rp", bufs=1) as rpp,
                        tc.tile_pool(name="ps_s", bufs=2, space="PSUM") as pss,
                        tc.tile_pool(name="ps_o", bufs=1, space="PSUM") as pso,
                    ):
                        for j in range(NJ):
                            kt_sb = ktp.tile([P, S], F32R, tag="kt")
                            nc.sync.dma_start(out=kt_sb, in_=kt_dram[j][:])
                            ps_oa = pso.tile([VW, SQ], F32, tag="oa")
                            ps_ob = pso.tile([VW, SQ], F32, tag="ob")
                            ets = {}

                            def scores_exp(kc, j=j, kt_sb=kt_sb, ets=ets):
                                for hh in (0, 1):
                                    ps_s = pss.tile(
                                        [P, SQ], F32, tag="s", name="ps_s"
                                    )
                                    lhsT = kt_sb[hh * 64:(hh + 1) * 64,
                                                 kc * P:(kc + 1) * P]
                                    for nh in range(2):
                                        nc.tensor.matmul(
                                            ps_s[:, nh * 512:(nh + 1) * 512],
                                            lhsT,
                                            qt[j][hh * 64:(hh + 1) * 64,
                                                  nh * 512:(nh + 1) * 512],
                                            tile_position=(hh * 64, 0),
                                        )
                                    et = expp.tile(
                                        [P, SQ], F32R, tag="e", name="et"
                                    )
                                    nc.scalar.activation(
                                        et, ps_s, Exp,
                                        bias=maskb_sb[:, kc:kc + 1], scale=1.0,
                                    )
                                    ets[(kc, hh)] = et

                            def pv(kc, j=j, ets=ets, ps_oa=ps_oa, ps_ob=ps_ob):
                                for hh, ps_o in ((0, ps_oa), (1, ps_ob)):
                                    vh = v_sb[:, kc, 2 * j + hh, :]  # [128,65]
                                    et = ets.pop((kc, hh))
                                    for nh in range(2):
                                        nc.tensor.matmul(
                                            ps_o[:, nh * 512:(nh + 1) * 512],
                                            vh,
                                            et[:, nh * 512:(nh + 1) * 512],
                                            start=(kc == 0),
                                            stop=(kc == KC - 1),
                                        )

                            scores_exp(0)
                            for kc in range(1, KC):
                                scores_exp(kc)
                                pv(kc - 1)
                            pv(KC - 1)
                            # normalize: row 64 of ps_o* is l
                            rrow = rpp.tile([VW, 2, SQ], F32, tag="rr")
                            nc.vector.reciprocal(
                                rrow[DH:VW, 0, :], ps_oa[DH:VW, :]
                            )
                            nc.vector.reciprocal(
                                rrow[DH:VW, 1, :], ps_ob[DH:VW, :]
                            )
                            L_sb = lbp.tile([P, SQ], F32, tag="L")
                            for hh in range(2):
                                rd = rdp.tile([1, SQ], F32, tag="rd", name="rd")
                                nc.sync.dma_start(
                                    out=rd, in_=rrow[DH:VW, hh, :]
                                )
                                rd_b = bass.AP(
                                    tensor=rd.tensor, offset=rd.offset,
                                    ap=[[0, 64], rd.ap[-1]],
                                )
                                nc.sync.dma_start(
                                    out=L_sb[hh * 64:(hh + 1) * 64, :],
                                    in_=rd_b,
                                )
                            nc.vector.tensor_mul(
                                ot[j][0:64, :], ps_oa[0:DH, :], L_sb[0:64, :]
                            )
                            tmpb = lbp.tile([64, SQ], F32R, tag="tmpb")
                            nc.vector.tensor_copy(tmpb, ps_ob[0:DH, :])
                            nc.gpsimd.dma_start(
                                out=ot[j][64:128, :], in_=tmpb
                            )
                            nc.vector.tensor_mul(
                                ot[j][64:128, :], ot[j][64:128, :],
                                L_sb[64:128, :],
                            )

                    # ---- output projection -------------------------------
                    with (
                        tc.tile_pool(name="wop", bufs=4) as wop,
                        tc.tile_pool(name="ytp", bufs=3) as ytp,
                        tc.tile_pool(name="py", bufs=2, space="PSUM") as pyp,
                    ):
                        yt_ch = yT.rearrange("(c p) s -> c p s", p=P)
                        for dc in range(DC):
                            ps = pyp.tile([P, SQ], F32, tag="py")
                            for j in range(NJ):
                                wo_t = wop.tile([P, P], F32R, tag="wo")
                                nc.scalar.dma_start(
                                    out=wo_t,
                                    in_=wo[j * P:(j + 1) * P,
                                           dc * P:(dc + 1) * P],
                                )
                                for nh in range(2):
                                    nc.tensor.matmul(
                                        ps[:, nh * 512:(nh + 1) * 512],
                                        wo_t,
                                        ot[j][:, nh * 512:(nh + 1) * 512],
                                        start=(j == 0),
                                        stop=(j == NJ - 1),
                                    )
                            yt_sb = ytp.tile([P, SQ], F32, tag="yt")
                            nc.vector.tensor_scalar(
                                yt_sb, ps, bo2_sb[:, dc:dc + 1], None, AOp.add
                            )
                            nc.gpsimd.dma_start(out=yt_ch[dc], in_=yt_sb)

    nc.compile()
    return nc


def kernel(x_Q, x_K, x_V, src_batch_lens, Wq, bq, Wk, bk, Wv, bv, Wo, bo):
    x_Q = np.asarray(x_Q, dtype=np.float32)
    x_K = np.asarray(x_K, dtype=np.float32)
    x_V = np.asarray(x_V, dtype=np.float32)
    lens = np.asarray(src_batch_lens)
    Wq = np.ascontiguousarray(np.asarray(Wq, dtype=np.float32))
    Wk = np.ascontiguousarray(np.asarray(Wk, dtype=np.float32))
    Wv = np.ascontiguousarray(np.asarray(Wv, dtype=np.float32))
    Wo = np.ascontiguousarray(np.asarray(Wo, dtype=np.float32))
    bq = np.asarray(bq, dtype=np.float32)
    bv = np.asarray(bv, dtype=np.float32)
    bo = np.asarray(bo, dtype=np.float32)

    if "nc" not in _CACHE:
        _CACHE["nc"] = build_bass()
    nc = _CACHE["nc"]

    bo2_full = (bv @ Wo + bo).astype(np.float32)
    bo2 = np.ascontiguousarray(bo2_full.reshape(DC, P).T)
    bq8 = np.ascontiguousarray((bq / 8.0).reshape(NJ, P).T)

    in_maps = []
    for c in range(8):
        b, hh = c // 2, c % 2
        q0 = hh * SQ
        k_idx = np.arange(S)
        mvec = np.where(k_idx < int(lens[b]), 0.0, MASK_NEG).astype(np.float32)
        in_maps.append({
            "xqT": np.ascontiguousarray(x_Q[b, q0:q0 + SQ, :].T),
            "xkT": np.ascontiguousarray(x_K[b].T),
            "xvT": np.ascontiguousarray(x_V[b].T),
            "wq": Wq, "wk": Wk, "wv": Wv, "wo": Wo,
            "bq8": bq8, "bo2": bo2,
            "vones": np.ones((P, SC * H), np.float32),
            "maskb": np.ascontiguousarray(mvec.reshape(KC, P).T),
        })

    res = run_bass_kernel_spmd(nc, in_maps, core_ids=list(range(8)))

    out = np.empty((B, S, D), dtype=np.float32)
    for c in range(8):
        b, hh = c // 2, c % 2
        q0 = hh * SQ
        out[b, q0:q0 + SQ, :] = res.results[c]["yT"].T
    return out
